# revision 25
# baseline (speedup 1.0000x reference)
"""Trainium2 Bass kernel for nn_Attention_3126736192307.

Causal multi-head attention with RoPE: B=2, S=2048, H=2048, 16 heads x 128.

Sharding (tensor parallel over heads, 8 cores, 2 heads each):
  - Wq/Wk/Wv column-split (per-head), Wo row-split; each core computes a
    partial [B*S, H] output; the host sums the 8 partials (row-parallel
    unshard) - no on-device collectives needed.

Per-core dataflow (all matmuls transpose-free by construction):
  - Host pre-transposes: X.T [H, T], WqT/WkT [H, 256] (head-dim permuted so
    RoPE's rotate_half becomes an intra-quadrant stream_shuffle), WvT [H, 256],
    WoT [256, H], cos/sin [128, T] feature-major bf16 (sin sign-folded).
  - Phase 1 per 512-token chunk: whole chunk's X.T tile staged in SBUF;
    v-pass (token-major PSUM) runs BEFORE the q/k-pass so the PE has
    bank-independent work at every chunk boundary; q/k evict through
    ScalarE casts (releases the PSUM banks fast) into bf16, then RoPE on
    DVE in bf16 (2x mode for mul/add).
  - Phase 2 per (b, h, i-chunk): scores.T [j,i] = k.T (lhsT) @ q.T; exp on
    ScalarE (no max subtraction - scores are ~N(0,1) after the 1/sqrt(hd)
    scale); causal block skipping + 0/1 mask multiply on diagonal-crossing
    tiles; column sums via ones-matmul on TensorE; AV accumulation in PSUM;
    normalization folded into the PSUM->SBUF eviction. Wo-partial PSUM
    eviction round-robins between DVE and ScalarE; output rows batched
    into [128, H] tiles and stored with one contiguous DMA per row-tile.

Matmuls run in bf16 (1 PE cycle/row; fp32 is 4x, and fp32r's fused
weight-load encoding can't carry the 2 semaphore waits Tile emits).
"""

import os
import sys

for _p in ("/opt/trn_rl_repo", "/root/.axon_site/_ro/trn_rl_repo"):
    if os.path.isdir(_p) and _p not in sys.path:
        sys.path.append(_p)

from contextlib import ExitStack

import ml_dtypes
import numpy as np

import concourse.bass as bass
import concourse.bacc as bacc
import concourse.tile as tile
from concourse import mybir
from concourse.bass_utils import run_bass_kernel_spmd

B, S, H, NH = 2, 2048, 2048, 16
HD = 128
NCORES = 8
HPC = NH // NCORES            # heads per core = 2
M = HPC * HD                  # 256 output channels per core
SCALE = HD ** -0.5
P = 128                       # partitions
NKT = H // P                  # 16 contraction tiles for projections

F32 = mybir.dt.float32

# head-dim permutation: interleave halves at 16 granularity so the RoPE
# partner (d <-> d+64) sits 16 partitions away inside one 32-part quadrant
PERM = np.concatenate([np.arange(16 * m, 16 * m + 16) + (64 if odd else 0)
                       for m in range(4) for odd in (0, 1)])
SWAP_MASK = [i ^ 16 for i in range(32)]


BF16 = ml_dtypes.bfloat16


def build_masks(tchunk):
    """0/1 keep-masks for the R diagonal-crossing j-tiles of each i-chunk."""
    r = tchunk // P
    m = np.zeros((r, P, tchunk), np.float32)
    il = np.arange(tchunk)
    for ri in range(r):
        for jl in range(P):
            m[ri, jl, :] = (P * ri + jl <= il).astype(np.float32)
    return m


def build_nc(s=S, b=B, tchunk=512, mm_dtype=mybir.dt.bfloat16):
    t = b * s
    tchunk = min(tchunk, t)
    nch = t // tchunk             # phase-1 token chunks
    ich = s // tchunk             # attention i-chunks per batch
    r_mask = tchunk // P          # diagonal-crossing tiles per i-chunk
    ntt = t // P                  # token tiles
    nvp = tchunk // P             # v psum tiles per chunk

    FR = mm_dtype

    nc = bacc.Bacc("TRN2", target_bir_lowering=False, debug=False)

    xt = nc.declare_dram_parameter("xt", [H, t], FR, isOutput=False)
    wqt = nc.declare_dram_parameter("wqt", [H, M], FR, isOutput=False)
    wkt = nc.declare_dram_parameter("wkt", [H, M], FR, isOutput=False)
    wvt = nc.declare_dram_parameter("wvt", [H, M], FR, isOutput=False)
    wot = nc.declare_dram_parameter("wot", [M, H], FR, isOutput=False)
    cost = nc.declare_dram_parameter("cost", [HD, t], FR, isOutput=False)
    sint = nc.declare_dram_parameter("sint", [HD, t], FR, isOutput=False)
    masks = nc.declare_dram_parameter("masks", [r_mask, P, tchunk], FR,
                                      isOutput=False)
    out = nc.declare_dram_parameter("out", [t, H], FR, isOutput=True)

    with tile.TileContext(nc) as tc, ExitStack() as ctx:
        persist = ctx.enter_context(tc.tile_pool(name="persist", bufs=1))

        # persistent activations
        qr = [persist.tile([P, t], FR, tag=f"qr{h}", name=f"qr{h}") for h in range(HPC)]
        kr = [persist.tile([P, t], FR, tag=f"kr{h}", name=f"kr{h}") for h in range(HPC)]
        vv = persist.tile([P, ntt, M], FR, tag="vv")   # v[tt*128+p, d]
        ones_s = persist.tile([P, P], FR, tag="ones")
        nc.vector.memset(ones_s[:], 1.0)
        # allocated up-front (fresh SBUF -> no reuse waits on their DMAs)
        mask_s = persist.tile([P, r_mask, tchunk], FR, tag="masks")
        wo_s = persist.tile([P, HPC, H], FR, tag="wo")
        # output row-tile staging: [128 tokens, H]; one contiguous DMA per tile
        ev_pool = ctx.enter_context(tc.tile_pool(name="evp", bufs=4))
        # whole-kernel 2-bank PSUM tiles: phase-1 q/k accumulator pairs and
        # attention score tiles rotate through the same two slots (A, B)
        ab_pool = ctx.enter_context(tc.tile_pool(name="ab", bufs=1, space="PSUM"))

        # ---------------- phase 1: projections + rope -----------------
        with (
            tc.tile_pool(name="csin", bufs=2) as csin_pool,
            tc.tile_pool(name="xtp", bufs=2) as xt_pool,
            tc.tile_pool(name="rtmp", bufs=3) as rtmp_pool,
            tc.tile_pool(name="qkbf", bufs=2) as qkbf_pool,
            tc.tile_pool(name="wts", bufs=1) as wts_pool,
            tc.tile_pool(name="p1v", bufs=1, space="PSUM") as p1v,
        ):
            wq_s = wts_pool.tile([P, NKT, M], FR, tag="wq")
            wk_s = wts_pool.tile([P, NKT, M], FR, tag="wk")
            wv_s = wts_pool.tile([P, NKT, M], FR, tag="wv")
            KG = 4                       # k-tiles per DMA group

            # wv first (v-pass consumes it first) on the scalar HWDGE queue;
            # wq/wk on the gpsimd SWDGE queue - three queues in parallel
            # with SP's xt loads.
            for g in range(NKT // KG):
                gsl = slice(g * KG * P, (g + 1) * KG * P)
                nc.scalar.dma_start(
                    out=wv_s[:, g * KG:(g + 1) * KG, :],
                    in_=wvt[gsl, :].rearrange("(k p) m -> p k m", p=P))

            # preload the Exp activation table while everything else loads
            # (otherwise the 1.3us ACT_TABLE_LOAD lands on the phase-2
            # critical path at the first real exp)
            dum = rtmp_pool.tile([P, 8], F32, tag="dum")
            nc.scalar.activation(out=dum[:], in_=ones_s[:, :8],
                                 func=mybir.ActivationFunctionType.Exp)
            for g in range(NKT // KG):
                gsl = slice(g * KG * P, (g + 1) * KG * P)
                for w_s, wsrc in ((wq_s, wqt), (wk_s, wkt)):
                    nc.gpsimd.dma_start(
                        out=w_s[:, g * KG:(g + 1) * KG, :],
                        in_=wsrc[gsl, :].rearrange("(k p) m -> p k m", p=P))
            for c in range(nch):
                tsl = slice(c * tchunk, (c + 1) * tchunk)
                xtc = xt_pool.tile([P, NKT, tchunk], FR, tag="xt",
                                   name=f"xtc_{c}")
                if c == 0:
                    # split the first group into single k-tiles so the very
                    # first v matmul can start as early as possible
                    for kt in range(KG):
                        ksl = slice(kt * P, (kt + 1) * P)
                        nc.sync.dma_start(
                            out=xtc[:, kt:kt + 1, :],
                            in_=xt[ksl, tsl].rearrange("(k p) t -> p k t",
                                                       p=P))
                    grange = [1, 2, 3]
                else:
                    grange = range(NKT // KG)
                for g in grange:
                    gsl = slice(g * KG * P, (g + 1) * KG * P)
                    nc.sync.dma_start(
                        out=xtc[:, g * KG:(g + 1) * KG, :],
                        in_=xt[gsl, tsl].rearrange("(k p) t -> p k t", p=P))
                if c == 4:
                    # phase-2 constants off the startup-critical DMA window
                    nc.gpsimd.dma_start(out=mask_s[:],
                                        in_=masks.rearrange("r p n -> p r n"))
                if c == 6:
                    nc.gpsimd.dma_start(
                        out=wo_s[:],
                        in_=wot.rearrange("(mt p) o -> p mt o", p=P))

                cos_t = csin_pool.tile([P, tchunk], FR, tag="cos")
                sin_t = csin_pool.tile([P, tchunk], FR, tag="sin")
                nc.gpsimd.dma_start(out=cos_t[:], in_=cost[:, tsl])
                nc.gpsimd.dma_start(out=sin_t[:], in_=sint[:, tsl])

                # ---- v-pass: bank-independent PE work covering the A/B
                # handoff at every chunk boundary ----
                v_ps = [p1v.tile([P, M], F32, tag=f"p1v{i}",
                                 name=f"p1v{i}_{c}") for i in range(nvp)]
                for kt in range(NKT):
                    fl = dict(start=(kt == 0), stop=(kt == NKT - 1))
                    for ts_ in range(nvp):
                        ssl = slice(ts_ * P, (ts_ + 1) * P)
                        nc.tensor.matmul(v_ps[ts_][:], xtc[:, kt, ssl],
                                         wv_s[:, kt, :], **fl)
                # v eviction first in the DVE queue: frees the v banks while
                # the qk-pass still runs
                for ts_ in range(nvp):
                    nc.vector.tensor_copy(out=vv[:, c * nvp + ts_, :],
                                          in_=v_ps[ts_][:])

                # ---- qk-pass ----
                q_ps = ab_pool.tile([P, HPC, 512], F32, tag="A", name=f"qps_{c}")
                k_ps = ab_pool.tile([P, HPC, 512], F32, tag="B", name=f"kps_{c}")
                qk_ps = [q_ps[:, 0, :tchunk], q_ps[:, 1, :tchunk],
                         k_ps[:, 0, :tchunk], k_ps[:, 1, :tchunk]]
                for kt in range(NKT):
                    fl = dict(start=(kt == 0), stop=(kt == NKT - 1))
                    for wi, w_s in enumerate((wq_s, wk_s)):
                        for h in range(HPC):
                            msl = slice(h * P, (h + 1) * P)
                            nc.tensor.matmul(qk_ps[wi * HPC + h][:],
                                             w_s[:, kt, msl],
                                             xtc[:, kt, :], **fl)

                # rope eviction: ScalarE casts the psum to bf16 (this is
                # what releases the A/B banks - fast), DVE then does
                # dest = qc*cos + shuffle(qc)*sin_eff in bf16 (2x mode)
                for wi, dest in ((0, qr), (1, kr)):
                    for h in range(HPC):
                        ps = qk_ps[wi * HPC + h]
                        qc = qkbf_pool.tile([P, tchunk], FR,
                                            tag=f"qc{wi}{h}",
                                            name=f"qc{wi}{h}_{c}")
                        nc.scalar.activation(
                            out=qc[:], in_=ps,
                            func=mybir.ActivationFunctionType.Copy)
                        shuf = rtmp_pool.tile([P, tchunk], FR, tag="shuf",
                                              name=f"shuf{wi}{h}_{c}")
                        dst = dest[h][:, tsl]
                        nc.vector.stream_shuffle(out=shuf[:], in_=qc[:],
                                                 mask=SWAP_MASK)
                        nc.vector.tensor_mul(out=dst, in0=qc[:], in1=cos_t[:])
                        nc.vector.tensor_mul(out=shuf[:], in0=shuf[:],
                                             in1=sin_t[:])
                        nc.vector.tensor_add(out=dst, in0=dst, in1=shuf[:])

        # -------- phase 2+3: attention with interleaved output proj -------
        # Software-pipelined: QK for tile jt+1 issues before colsum/AV of jt,
        # and both heads' exp runs as ONE wide ACT op over a 2-bank PSUM
        # tile, so ACT latency never blocks the PE stream.
        with (
            tc.tile_pool(name="outp", bufs=1) as out_pool,
            tc.tile_pool(name="exps", bufs=10) as exps_pool,
            tc.tile_pool(name="rcp", bufs=2) as rcp_pool,
            tc.tile_pool(name="p2cs", bufs=1, space="PSUM") as p2cs,
            tc.tile_pool(name="p2av", bufs=1, space="PSUM") as p2av,
        ):
            outT = [out_pool.tile([P, t], FR, tag=f"outT{h}", name=f"outT{h}")
                    for h in range(HPC)]
            ev_rr = [0]   # wo-eviction round-robin counter

            def drain_one(pend):
                (pes, plo, pw, pfl, pjt, ctx_) = pend.pop(0)
                (bb_, cs_l, av_l, isl_, c_) = ctx_
                for h in range(HPC):
                    nc.tensor.matmul(cs_l[h][:, plo:], ones_s[:],
                                     pes[:, h, :pw], **pfl)
                    nc.tensor.matmul(av_l[h][:, plo:],
                                     vv[:, bb_ * (s // P) + pjt,
                                        h * P:(h + 1) * P],
                                     pes[:, h, :pw], **pfl)
                if not pfl["stop"]:
                    return
                # chunk epilogue: normalize + output projection
                for h in range(HPC):
                    rcp = rcp_pool.tile([P, tchunk], F32, tag="rcp",
                                        name=f"rcp{h}_{bb_}_{c_}")
                    nc.vector.reciprocal_approx_fast(out=rcp[:], in_=cs_l[h][:])
                    nc.vector.tensor_mul(out=outT[h][:, isl_], in0=av_l[h][:],
                                         in1=rcp[:])
                wo_pools = [p2cs, p2cs, p2av, p2av]
                wo_tags = ["cs0", "cs1", "av0", "av1"]
                wi_ = 0
                for tt_ in range(tchunk // P):
                    tt0 = isl_.start + tt_ * P
                    ttsl = slice(tt0, tt0 + P)
                    ev = ev_pool.tile([P, H], FR, tag="ev",
                                      name=f"ev_{tt0}")
                    for oc in range(H // 512):
                        osl = slice(oc * 512, (oc + 1) * 512)
                        ps = wo_pools[wi_ % 4].tile(
                            [P, 512], F32, tag=wo_tags[wi_ % 4],
                            name=f"wo_{tt0}_{oc}")
                        wi_ += 1
                        for h in range(HPC):
                            nc.tensor.matmul(ps[:],
                                             outT[h][:, ttsl],
                                             wo_s[:, h, osl],
                                             start=(h == 0),
                                             stop=(h == HPC - 1))
                        # eviction split: 2 of 3 on DVE, 1 of 3 on ScalarE
                        # (exp keeps ScalarE on the QK critical chain)
                        if ev_rr[0] % 3 < 2:
                            nc.vector.tensor_copy(out=ev[:, osl], in_=ps[:])
                        else:
                            nc.scalar.activation(
                                out=ev[:, osl], in_=ps[:],
                                func=mybir.ActivationFunctionType.Copy)
                        ev_rr[0] += 1
                    nc.sync.dma_start(out=out[ttsl, :], in_=ev[:])

            pend = []
            for bb in range(b):
                # chunk order is causality-free: start deep (warmup), keep
                # the shallow chunks sandwiched, finish deep (drain-rich)
                for c in (2, 0, 1, 3) if ich == 4 else reversed(range(ich)):
                    isl = slice(bb * s + c * tchunk, bb * s + (c + 1) * tchunk)
                    njt = r_mask * (c + 1)   # visible j-tiles
                    cs_ps = [p2cs.tile([P, tchunk], F32, tag=f"cs{h}",
                                       name=f"cs{h}_{bb}_{c}") for h in range(HPC)]
                    av_ps = [p2av.tile([P, tchunk], F32, tag=f"av{h}",
                                       name=f"av{h}_{bb}_{c}") for h in range(HPC)]
                    cctx = (bb, cs_ps, av_ps, isl, c)
                    for jt in range(njt):
                        jsl = slice(bb * s + jt * P, bb * s + (jt + 1) * P)
                        ri = jt - r_mask * c
                        lo = max(ri, 0) * P
                        w = tchunk - lo
                        csl = slice(isl.start + lo, isl.stop)
                        fl = dict(start=(jt == 0), stop=(jt == njt - 1))
                        sc = ab_pool.tile([P, HPC, 512], F32,
                                          tag=("A", "B")[jt % 2],
                                          name=f"sc_{bb}_{c}_{jt}")
                        for h in range(HPC):
                            nc.tensor.matmul(sc[:, h, :w], kr[h][:, jsl],
                                             qr[h][:, csl],
                                             start=True, stop=True)
                        es = exps_pool.tile([P, HPC, tchunk], FR, tag="es",
                                            name=f"es_{bb}_{c}_{jt}")
                        nc.scalar.activation(out=es[:, :, :w], in_=sc[:, :, :w],
                                             func=mybir.ActivationFunctionType.Exp,
                                             scale=float(SCALE))
                        if ri >= 0:  # diagonal-crossing tile
                            mb = mask_s[:, ri, lo:].unsqueeze(1).broadcast_to(
                                [P, HPC, w])
                            # mask on the otherwise-idle Pool engine; the
                            # ~5-tile drain pipeline absorbs its latency
                            nc.gpsimd.tensor_mul(out=es[:, :, :w],
                                                 in0=es[:, :, :w], in1=mb)
                        pend.append((es, lo, w, fl, jt, cctx))
                        if len(pend) > 4:
                            drain_one(pend)
            while pend:
                drain_one(pend)

    nc.compile()
    return nc


def make_in_maps(hidden_states, cos, sin, Wq, Wk, Wv, Wo, s=S, b=B, tchunk=512):
    t = b * s
    tchunk = min(tchunk, t)
    hs = np.asarray(hidden_states, np.float32).reshape(t, H)
    xt = np.ascontiguousarray(hs.T)
    cos2 = np.asarray(cos, np.float32).reshape(s, HD)
    sin2 = np.asarray(sin, np.float32).reshape(s, HD)
    cosP = np.ascontiguousarray(np.tile(cos2[:, PERM].T, (1, b))).astype(BF16)
    sign = np.where(PERM < 64, -1.0, 1.0).astype(np.float32)[:, None]
    sinP = np.ascontiguousarray(np.tile(sin2[:, PERM].T * sign, (1, b))).astype(BF16)
    masks_bf = build_masks(tchunk).astype(BF16)
    xt_bf = xt.astype(BF16)
    Wq, Wk, Wv, Wo = (np.asarray(w, np.float32) for w in (Wq, Wk, Wv, Wo))

    in_maps = []
    for c in range(NCORES):
        rows = np.concatenate([(HPC * c + hh) * HD + PERM for hh in range(HPC)])
        sl = slice(c * M, (c + 1) * M)
        in_maps.append({
            "xt": xt_bf,
            "wqt": np.ascontiguousarray(Wq[rows, :].T).astype(BF16),
            "wkt": np.ascontiguousarray(Wk[rows, :].T).astype(BF16),
            "wvt": np.ascontiguousarray(Wv[sl, :].T).astype(BF16),
            "wot": np.ascontiguousarray(Wo[:, sl].T).astype(BF16),
            "cost": cosP,
            "sint": sinP,
            "masks": masks_bf,
        })
    return in_maps


_CACHED_NC = None
_LAST_RESULTS = None


def kernel(hidden_states, cos, sin, Wq, Wk, Wv, Wo):
    global _CACHED_NC, _LAST_RESULTS
    in_maps = make_in_maps(hidden_states, cos, sin, Wq, Wk, Wv, Wo)
    if _CACHED_NC is None:
        _CACHED_NC = build_nc()
    res = run_bass_kernel_spmd(_CACHED_NC, in_maps, core_ids=list(range(NCORES)))
    _LAST_RESULTS = res
    acc = np.zeros((B * S, H), np.float32)
    for r in res.results:
        acc += r["out"].astype(np.float32)
    return acc.reshape(B, S, H)


# revision 27
# speedup vs baseline: 1.0069x; 1.0069x over previous
"""Trainium2 Bass kernel for nn_Attention_3126736192307.

Causal multi-head attention with RoPE: B=2, S=2048, H=2048, 16 heads x 128.

Sharding (tensor parallel over heads, 8 cores, 2 heads each):
  - Wq/Wk/Wv column-split (per-head), Wo row-split; each core computes a
    partial [B*S, H] output; the host sums the 8 partials (row-parallel
    unshard) - no on-device collectives needed.

Per-core dataflow (all matmuls transpose-free by construction):
  - Host pre-transposes: X.T [H, T], WqT/WkT [H, 256] (head-dim permuted so
    RoPE's rotate_half becomes an intra-quadrant stream_shuffle), WvT [H, 256],
    WoT [256, H], cos/sin [128, T] feature-major bf16 (sin sign-folded).
  - Phase 1 per 512-token chunk: whole chunk's X.T tile staged in SBUF;
    v-pass (token-major PSUM) runs BEFORE the q/k-pass so the PE has
    bank-independent work at every chunk boundary; q/k evict through
    ScalarE casts (releases the PSUM banks fast) into bf16, then RoPE on
    DVE in bf16 (2x mode for mul/add).
  - Phase 2 per (b, h, i-chunk): scores.T [j,i] = k.T (lhsT) @ q.T; exp on
    ScalarE (no max subtraction - scores are ~N(0,1) after the 1/sqrt(hd)
    scale); causal block skipping + 0/1 mask multiply on diagonal-crossing
    tiles; column sums via ones-matmul on TensorE; AV accumulation in PSUM;
    normalization folded into the PSUM->SBUF eviction. Wo-partial PSUM
    eviction round-robins between DVE and ScalarE; output rows batched
    into [128, H] tiles and stored with one contiguous DMA per row-tile.

Matmuls run in bf16 (1 PE cycle/row; fp32 is 4x, and fp32r's fused
weight-load encoding can't carry the 2 semaphore waits Tile emits).
"""

import os
import sys

for _p in ("/opt/trn_rl_repo", "/root/.axon_site/_ro/trn_rl_repo"):
    if os.path.isdir(_p) and _p not in sys.path:
        sys.path.append(_p)

from contextlib import ExitStack

import ml_dtypes
import numpy as np

import concourse.bass as bass
import concourse.bacc as bacc
import concourse.tile as tile
from concourse import mybir
from concourse.bass_utils import run_bass_kernel_spmd

B, S, H, NH = 2, 2048, 2048, 16
HD = 128
NCORES = 8
HPC = NH // NCORES            # heads per core = 2
M = HPC * HD                  # 256 output channels per core
SCALE = HD ** -0.5
P = 128                       # partitions
NKT = H // P                  # 16 contraction tiles for projections

F32 = mybir.dt.float32

# head-dim permutation: interleave halves at 16 granularity so the RoPE
# partner (d <-> d+64) sits 16 partitions away inside one 32-part quadrant
PERM = np.concatenate([np.arange(16 * m, 16 * m + 16) + (64 if odd else 0)
                       for m in range(4) for odd in (0, 1)])
SWAP_MASK = [i ^ 16 for i in range(32)]


BF16 = ml_dtypes.bfloat16


def build_masks(tchunk):
    """0/1 keep-masks for the R diagonal-crossing j-tiles of each i-chunk."""
    r = tchunk // P
    m = np.zeros((r, P, tchunk), np.float32)
    il = np.arange(tchunk)
    for ri in range(r):
        for jl in range(P):
            m[ri, jl, :] = (P * ri + jl <= il).astype(np.float32)
    return m


def build_nc(s=S, b=B, tchunk=512, mm_dtype=mybir.dt.bfloat16):
    t = b * s
    tchunk = min(tchunk, t)
    nch = t // tchunk             # phase-1 token chunks
    ich = s // tchunk             # attention i-chunks per batch
    r_mask = tchunk // P          # diagonal-crossing tiles per i-chunk
    ntt = t // P                  # token tiles
    nvp = tchunk // P             # v psum tiles per chunk

    FR = mm_dtype

    nc = bacc.Bacc("TRN2", target_bir_lowering=False, debug=False)

    xt = nc.declare_dram_parameter("xt", [H, t], FR, isOutput=False)
    wqt = nc.declare_dram_parameter("wqt", [H, M], FR, isOutput=False)
    wkt = nc.declare_dram_parameter("wkt", [H, M], FR, isOutput=False)
    wvt = nc.declare_dram_parameter("wvt", [H, M], FR, isOutput=False)
    wot = nc.declare_dram_parameter("wot", [M, H], FR, isOutput=False)
    cost = nc.declare_dram_parameter("cost", [HD, t], FR, isOutput=False)
    sint = nc.declare_dram_parameter("sint", [HD, t], FR, isOutput=False)
    masks = nc.declare_dram_parameter("masks", [r_mask, P, tchunk], FR,
                                      isOutput=False)
    out = nc.declare_dram_parameter("out", [t, H], FR, isOutput=True)

    with tile.TileContext(nc) as tc, ExitStack() as ctx:
        persist = ctx.enter_context(tc.tile_pool(name="persist", bufs=1))

        # persistent activations
        qr = [persist.tile([P, t], FR, tag=f"qr{h}", name=f"qr{h}") for h in range(HPC)]
        kr = [persist.tile([P, t], FR, tag=f"kr{h}", name=f"kr{h}") for h in range(HPC)]
        vv = persist.tile([P, ntt, M], FR, tag="vv")   # v[tt*128+p, d]
        ones_s = persist.tile([P, P], FR, tag="ones")
        nc.vector.memset(ones_s[:], 1.0)
        # allocated up-front (fresh SBUF -> no reuse waits on their DMAs)
        mask_s = persist.tile([P, r_mask, tchunk], FR, tag="masks")
        wo_s = persist.tile([P, HPC, H], FR, tag="wo")
        # output row-tile staging: [128 tokens, H]; one contiguous DMA per tile
        ev_pool = ctx.enter_context(tc.tile_pool(name="evp", bufs=4))
        # whole-kernel 2-bank PSUM tiles: phase-1 q/k accumulator pairs and
        # attention score tiles rotate through the same two slots (A, B)
        ab_pool = ctx.enter_context(tc.tile_pool(name="ab", bufs=1, space="PSUM"))

        # ---------------- phase 1: projections + rope -----------------
        with (
            tc.tile_pool(name="csin", bufs=2) as csin_pool,
            tc.tile_pool(name="xtp", bufs=2) as xt_pool,
            tc.tile_pool(name="rtmp", bufs=3) as rtmp_pool,
            tc.tile_pool(name="qkbf", bufs=2) as qkbf_pool,
            tc.tile_pool(name="wts", bufs=1) as wts_pool,
            tc.tile_pool(name="p1v", bufs=1, space="PSUM") as p1v,
        ):
            wq_s = wts_pool.tile([P, NKT, M], FR, tag="wq")
            wk_s = wts_pool.tile([P, NKT, M], FR, tag="wk")
            wv_s = wts_pool.tile([P, NKT, M], FR, tag="wv")
            KG = 4                       # k-tiles per DMA group

            # wv first (v-pass consumes it first) on the scalar HWDGE queue;
            # wq/wk on the gpsimd SWDGE queue - three queues in parallel
            # with SP's xt loads.
            for g in range(NKT // KG):
                gsl = slice(g * KG * P, (g + 1) * KG * P)
                nc.scalar.dma_start(
                    out=wv_s[:, g * KG:(g + 1) * KG, :],
                    in_=wvt[gsl, :].rearrange("(k p) m -> p k m", p=P))
            # preload the Exp activation table while everything else loads
            # (otherwise the 1.3us ACT_TABLE_LOAD lands on the phase-2
            # critical path at the first real exp)
            dum = rtmp_pool.tile([P, 8], F32, tag="dum")
            nc.scalar.activation(out=dum[:], in_=ones_s[:, :8],
                                 func=mybir.ActivationFunctionType.Exp)
            for g in range(NKT // KG):
                gsl = slice(g * KG * P, (g + 1) * KG * P)
                for w_s, wsrc in ((wq_s, wqt), (wk_s, wkt)):
                    nc.gpsimd.dma_start(
                        out=w_s[:, g * KG:(g + 1) * KG, :],
                        in_=wsrc[gsl, :].rearrange("(k p) m -> p k m", p=P))
            for c in range(nch):
                tsl = slice(c * tchunk, (c + 1) * tchunk)
                xtc = xt_pool.tile([P, NKT, tchunk], FR, tag="xt",
                                   name=f"xtc_{c}")
                if c == 0:
                    # split the first group into single k-tiles so the very
                    # first v matmul can start as early as possible
                    for kt in range(KG):
                        ksl = slice(kt * P, (kt + 1) * P)
                        nc.sync.dma_start(
                            out=xtc[:, kt:kt + 1, :],
                            in_=xt[ksl, tsl].rearrange("(k p) t -> p k t",
                                                       p=P))
                    grange = [1, 2, 3]
                else:
                    grange = range(NKT // KG)
                for g in grange:
                    gsl = slice(g * KG * P, (g + 1) * KG * P)
                    nc.sync.dma_start(
                        out=xtc[:, g * KG:(g + 1) * KG, :],
                        in_=xt[gsl, tsl].rearrange("(k p) t -> p k t", p=P))
                if c == 4:
                    # phase-2 constants off the startup-critical DMA window
                    nc.gpsimd.dma_start(out=mask_s[:],
                                        in_=masks.rearrange("r p n -> p r n"))
                if c == 6:
                    nc.gpsimd.dma_start(
                        out=wo_s[:],
                        in_=wot.rearrange("(mt p) o -> p mt o", p=P))

                cos_t = csin_pool.tile([P, tchunk], FR, tag="cos")
                sin_t = csin_pool.tile([P, tchunk], FR, tag="sin")
                nc.gpsimd.dma_start(out=cos_t[:], in_=cost[:, tsl])
                nc.gpsimd.dma_start(out=sin_t[:], in_=sint[:, tsl])

                # ---- v-pass: bank-independent PE work covering the A/B
                # handoff at every chunk boundary ----
                v_ps = [p1v.tile([P, M], F32, tag=f"p1v{i}",
                                 name=f"p1v{i}_{c}") for i in range(nvp)]
                for kt in range(NKT):
                    fl = dict(start=(kt == 0), stop=(kt == NKT - 1))
                    for ts_ in range(nvp):
                        ssl = slice(ts_ * P, (ts_ + 1) * P)
                        nc.tensor.matmul(v_ps[ts_][:], xtc[:, kt, ssl],
                                         wv_s[:, kt, :], **fl)
                # v eviction first in the DVE queue: frees the v banks while
                # the qk-pass still runs
                for ts_ in range(nvp):
                    nc.vector.tensor_copy(out=vv[:, c * nvp + ts_, :],
                                          in_=v_ps[ts_][:])

                # ---- qk-pass ----
                q_ps = ab_pool.tile([P, HPC, 512], F32, tag="A", name=f"qps_{c}")
                k_ps = ab_pool.tile([P, HPC, 512], F32, tag="B", name=f"kps_{c}")
                qk_ps = [q_ps[:, 0, :tchunk], q_ps[:, 1, :tchunk],
                         k_ps[:, 0, :tchunk], k_ps[:, 1, :tchunk]]
                for kt in range(NKT):
                    fl = dict(start=(kt == 0), stop=(kt == NKT - 1))
                    for wi, w_s in enumerate((wq_s, wk_s)):
                        for h in range(HPC):
                            msl = slice(h * P, (h + 1) * P)
                            nc.tensor.matmul(qk_ps[wi * HPC + h][:],
                                             w_s[:, kt, msl],
                                             xtc[:, kt, :], **fl)

                # rope eviction: ScalarE casts the psum to bf16 (this is
                # what releases the A/B banks - fast), DVE then does
                # dest = qc*cos + shuffle(qc)*sin_eff in bf16 (2x mode)
                for wi, dest in ((0, qr), (1, kr)):
                    for h in range(HPC):
                        ps = qk_ps[wi * HPC + h]
                        qc = qkbf_pool.tile([P, tchunk], FR,
                                            tag=f"qc{wi}{h}",
                                            name=f"qc{wi}{h}_{c}")
                        nc.scalar.activation(
                            out=qc[:], in_=ps,
                            func=mybir.ActivationFunctionType.Copy)
                        shuf = rtmp_pool.tile([P, tchunk], FR, tag="shuf",
                                              name=f"shuf{wi}{h}_{c}")
                        dst = dest[h][:, tsl]
                        nc.vector.stream_shuffle(out=shuf[:], in_=qc[:],
                                                 mask=SWAP_MASK)
                        nc.vector.tensor_mul(out=dst, in0=qc[:], in1=cos_t[:])
                        nc.vector.tensor_mul(out=shuf[:], in0=shuf[:],
                                             in1=sin_t[:])
                        nc.vector.tensor_add(out=dst, in0=dst, in1=shuf[:])

        # -------- phase 2+3: attention with interleaved output proj -------
        # Software-pipelined: QK for tile jt+1 issues before colsum/AV of jt,
        # and both heads' exp runs as ONE wide ACT op over a 2-bank PSUM
        # tile, so ACT latency never blocks the PE stream.
        with (
            tc.tile_pool(name="outp", bufs=1) as out_pool,
            tc.tile_pool(name="exps", bufs=10) as exps_pool,
            tc.tile_pool(name="rcp", bufs=2) as rcp_pool,
            tc.tile_pool(name="p2cs", bufs=1, space="PSUM") as p2cs,
            tc.tile_pool(name="p2av", bufs=1, space="PSUM") as p2av,
        ):
            outT = [out_pool.tile([P, t], FR, tag=f"outT{h}", name=f"outT{h}")
                    for h in range(HPC)]
            ev_rr = [0]   # wo-eviction round-robin counter

            def drain_one(pend):
                (pes, plo, pw, pfl, pjt, ctx_) = pend.pop(0)
                (bb_, cs_l, av_l, isl_, c_) = ctx_
                for h in range(HPC):
                    nc.tensor.matmul(cs_l[h][:, plo:], ones_s[:],
                                     pes[:, h, :pw], **pfl)
                    nc.tensor.matmul(av_l[h][:, plo:],
                                     vv[:, bb_ * (s // P) + pjt,
                                        h * P:(h + 1) * P],
                                     pes[:, h, :pw], **pfl)
                if not pfl["stop"]:
                    return
                # chunk epilogue: normalize + output projection
                for h in range(HPC):
                    rcp = rcp_pool.tile([P, tchunk], F32, tag="rcp",
                                        name=f"rcp{h}_{bb_}_{c_}")
                    nc.vector.reciprocal_approx_fast(out=rcp[:], in_=cs_l[h][:])
                    nc.vector.tensor_mul(out=outT[h][:, isl_], in0=av_l[h][:],
                                         in1=rcp[:])
                wo_pools = [p2cs, p2cs, p2av, p2av]
                wo_tags = ["cs0", "cs1", "av0", "av1"]
                wi_ = 0
                for tt_ in range(tchunk // P):
                    tt0 = isl_.start + tt_ * P
                    ttsl = slice(tt0, tt0 + P)
                    ev = ev_pool.tile([P, H], FR, tag="ev",
                                      name=f"ev_{tt0}")
                    for oc in range(H // 512):
                        osl = slice(oc * 512, (oc + 1) * 512)
                        ps = wo_pools[wi_ % 4].tile(
                            [P, 512], F32, tag=wo_tags[wi_ % 4],
                            name=f"wo_{tt0}_{oc}")
                        wi_ += 1
                        for h in range(HPC):
                            nc.tensor.matmul(ps[:],
                                             outT[h][:, ttsl],
                                             wo_s[:, h, osl],
                                             start=(h == 0),
                                             stop=(h == HPC - 1))
                        # eviction split: 2 of 3 on DVE, 1 of 3 on ScalarE
                        # (exp keeps ScalarE on the QK critical chain)
                        if ev_rr[0] % 3 < 2:
                            nc.vector.tensor_copy(out=ev[:, osl], in_=ps[:])
                        else:
                            nc.scalar.activation(
                                out=ev[:, osl], in_=ps[:],
                                func=mybir.ActivationFunctionType.Copy)
                        ev_rr[0] += 1
                    nc.sync.dma_start(out=out[ttsl, :], in_=ev[:])

            pend = []
            for bb in range(b):
                # largest chunk first: the deep chunk keeps the drain
                # pipeline fed through warmup; order is causality-free
                for c in reversed(range(ich)):
                    isl = slice(bb * s + c * tchunk, bb * s + (c + 1) * tchunk)
                    njt = r_mask * (c + 1)   # visible j-tiles
                    cs_ps = [p2cs.tile([P, tchunk], F32, tag=f"cs{h}",
                                       name=f"cs{h}_{bb}_{c}") for h in range(HPC)]
                    av_ps = [p2av.tile([P, tchunk], F32, tag=f"av{h}",
                                       name=f"av{h}_{bb}_{c}") for h in range(HPC)]
                    cctx = (bb, cs_ps, av_ps, isl, c)
                    for jt in range(njt):
                        jsl = slice(bb * s + jt * P, bb * s + (jt + 1) * P)
                        ri = jt - r_mask * c
                        lo = max(ri, 0) * P
                        w = tchunk - lo
                        csl = slice(isl.start + lo, isl.stop)
                        fl = dict(start=(jt == 0), stop=(jt == njt - 1))
                        sc = ab_pool.tile([P, HPC, 512], F32,
                                          tag=("A", "B")[jt % 2],
                                          name=f"sc_{bb}_{c}_{jt}")
                        for h in range(HPC):
                            nc.tensor.matmul(sc[:, h, :w], kr[h][:, jsl],
                                             qr[h][:, csl],
                                             start=True, stop=True)
                        es = exps_pool.tile([P, HPC, tchunk], FR, tag="es",
                                            name=f"es_{bb}_{c}_{jt}")
                        nc.scalar.activation(out=es[:, :, :w], in_=sc[:, :, :w],
                                             func=mybir.ActivationFunctionType.Exp,
                                             scale=float(SCALE))
                        if ri >= 0:  # diagonal-crossing tile
                            mb = mask_s[:, ri, lo:].unsqueeze(1).broadcast_to(
                                [P, HPC, w])
                            # mask on the otherwise-idle Pool engine; the
                            # ~5-tile drain pipeline absorbs its latency
                            nc.gpsimd.tensor_mul(out=es[:, :, :w],
                                                 in0=es[:, :, :w], in1=mb)
                        pend.append((es, lo, w, fl, jt, cctx))
                        if len(pend) > 4:
                            drain_one(pend)
            while pend:
                drain_one(pend)

    nc.compile()
    return nc


def make_in_maps(hidden_states, cos, sin, Wq, Wk, Wv, Wo, s=S, b=B, tchunk=512):
    t = b * s
    tchunk = min(tchunk, t)
    hs = np.asarray(hidden_states, np.float32).reshape(t, H)
    xt = np.ascontiguousarray(hs.T)
    cos2 = np.asarray(cos, np.float32).reshape(s, HD)
    sin2 = np.asarray(sin, np.float32).reshape(s, HD)
    cosP = np.ascontiguousarray(np.tile(cos2[:, PERM].T, (1, b))).astype(BF16)
    sign = np.where(PERM < 64, -1.0, 1.0).astype(np.float32)[:, None]
    sinP = np.ascontiguousarray(np.tile(sin2[:, PERM].T * sign, (1, b))).astype(BF16)
    masks_bf = build_masks(tchunk).astype(BF16)
    xt_bf = xt.astype(BF16)
    Wq, Wk, Wv, Wo = (np.asarray(w, np.float32) for w in (Wq, Wk, Wv, Wo))

    in_maps = []
    for c in range(NCORES):
        rows = np.concatenate([(HPC * c + hh) * HD + PERM for hh in range(HPC)])
        sl = slice(c * M, (c + 1) * M)
        in_maps.append({
            "xt": xt_bf,
            "wqt": np.ascontiguousarray(Wq[rows, :].T).astype(BF16),
            "wkt": np.ascontiguousarray(Wk[rows, :].T).astype(BF16),
            "wvt": np.ascontiguousarray(Wv[sl, :].T).astype(BF16),
            "wot": np.ascontiguousarray(Wo[:, sl].T).astype(BF16),
            "cost": cosP,
            "sint": sinP,
            "masks": masks_bf,
        })
    return in_maps


_CACHED_NC = None
_LAST_RESULTS = None


def kernel(hidden_states, cos, sin, Wq, Wk, Wv, Wo):
    global _CACHED_NC, _LAST_RESULTS
    in_maps = make_in_maps(hidden_states, cos, sin, Wq, Wk, Wv, Wo)
    if _CACHED_NC is None:
        _CACHED_NC = build_nc()
    res = run_bass_kernel_spmd(_CACHED_NC, in_maps, core_ids=list(range(NCORES)))
    _LAST_RESULTS = res
    acc = np.zeros((B * S, H), np.float32)
    for r in res.results:
        acc += r["out"].astype(np.float32)
    return acc.reshape(B, S, H)

